# revision 41
# baseline (speedup 1.0000x reference)
"""DeformConv1d Trainium2 Bass kernel.

Problem: x[4,512,4096] f32, offsets[4,1,4090,7] f32, weight[512,512,7], bias[512]
  T[b,o,k]   = clamp(o + k + offsets[b,0,o,k], o, o+6)
  samp[b,c,o,k] = linear-interp of x[b,c,:] at T
  out[b,d,o] = sum_{c,k} samp[b,c,o,k] * weight[d,c,k] + bias[d]

Key identity: the clamp keeps every sample inside [o, o+6], so output o only
touches taps p in [o, o+7], and the interpolation weight of tap p is exactly
relu(1 - |p - T|).  With an o-tile of 121 the tap band is exactly 128 wide:

  out[o,d] = sum_{k, dp<128} S_k[dp, o] * Y[o0+dp, k, d] + bias[d]
    S_k[dp,o] = relu(1 - |(dp - (o-o0)) - c_k[o]|),  c_k[o] = clamp(k + off, 0, 6)
    Y[p,k,d]  = sum_c x[c, p] * weight[d, c, k]

Both stages are dense bf16 matmuls on the PE array (f32 PSUM accumulate).
S is built directly in matmul layout: c = clamp(k + off, 0, 6) is computed
once on-chip in f32, staged to DRAM as per-tile contiguous rows, expanded
across all 128 partitions by a 0-stride DMA, and turned into tap weights by
subtract/abs/relu on DVE+ACT.  All PSUM evictions are pinned to specific
engines to avoid FIFO head-of-line inversions.
Sharding: 8 cores = 4 batches x 2 halves of out_len (data parallel, no comm).
"""

import os
import sys

import ml_dtypes
import numpy as np

for _p in ("/opt/trn_rl_repo", os.path.expanduser("~/.axon_site/_ro/trn_rl_repo")):
    if os.path.isdir(_p) and _p not in sys.path:
        sys.path.insert(0, _p)

import concourse.mybir as mybir
import concourse.tile as tile
from concourse import bacc
from concourse.bass_utils import run_bass_kernel_spmd

B, CIN, COUT, L, K = 4, 512, 512, 4096, 7
OUT_LEN = 4090
HALF = 2045          # out positions per core (2 halves per batch)
OT = 121             # o-tile size -> tap band = OT + 7 = 128
TILES = 17           # 17 * 121 = 2057 >= 2045
OPAD = TILES * OT    # 2057 padded out positions per core
XW = (TILES - 1) * OT + 128  # 2064: rightmost x column any tile reads
P = 128
NCK = CIN // P       # 4 c-chunks
F32 = mybir.dt.float32
BF16 = mybir.dt.bfloat16

_prog_cache = {}


def _build_program():
    nc = bacc.Bacc("TRN2", target_bir_lowering=False, debug=False)

    xs_d = nc.dram_tensor("xs", [CIN, XW], BF16, kind="ExternalInput")
    wt_d = nc.dram_tensor("wt", [CIN, K, COUT], BF16, kind="ExternalInput")
    offsT_d = nc.dram_tensor("offsT", [K, OPAD], F32, kind="ExternalInput")
    bias_d = nc.dram_tensor("bias2", [1, COUT], BF16, kind="ExternalInput")
    diag_d = nc.dram_tensor("diag7", [P, 1024], F32, kind="ExternalInput")
    kcol_d = nc.dram_tensor("kcol", [K, 1], F32, kind="ExternalInput")
    onesb_d = nc.dram_tensor("onesb", [1, P], BF16, kind="ExternalInput")
    out_d = nc.dram_tensor("out", [OPAD, COUT], F32, kind="ExternalOutput")

    with tile.TileContext(nc) as tc:
        with (
            tc.tile_pool(name="const", bufs=1) as cpool,
            tc.tile_pool(name="cdram", bufs=1, space="DRAM") as dpool,
            tc.tile_pool(name="cbt", bufs=4) as cbpool,
            tc.tile_pool(name="stiles", bufs=3) as stpool,
            tc.tile_pool(name="ytiles", bufs=3) as ypool,
            tc.tile_pool(name="otiles", bufs=3) as opool,
            tc.tile_pool(name="psy", bufs=5, space="PSUM") as psy,
            tc.tile_pool(name="pso", bufs=3, space="PSUM") as pso,
        ):
            # ---- small constants / offsets first (ahead of bulk x/W DMA) ----
            kcol = cpool.tile([K, 1], F32)
            nc.gpsimd.dma_start(kcol[:], kcol_d[:])
            onesb = cpool.tile([1, P], BF16)
            nc.gpsimd.dma_start(onesb[:], onesb_d[:])
            bias_sb = cpool.tile([1, COUT], BF16)
            nc.gpsimd.dma_start(bias_sb[:], bias_d[:])
            offsT = cpool.tile([K, OPAD], F32)
            nc.gpsimd.dma_start(offsT[:], offsT_d[:])
            diag7 = cpool.tile([P, 1024], F32)
            nc.gpsimd.dma_start(diag7[:], diag_d[:])

            # c[k, o] = clamp(k + off[k, o], 0, 6), staged to DRAM so a
            # 0-stride DMA can expand it across partitions per tile
            cexp = cpool.tile([K, OPAD], F32)
            nc.vector.tensor_scalar(
                cexp[:], offsT[:], kcol[:], 0.0,
                mybir.AluOpType.add, mybir.AluOpType.max,
            )
            nc.vector.tensor_scalar(
                cexp[:], cexp[:], 6.0, None, mybir.AluOpType.min,
            )
            # stage c to DRAM; per tile a small [1, 1024] row is DMA'd back
            # and expanded across partitions by a K=1 ones-matmul on the PE.
            c_dram = dpool.tile([K, OPAD], F32)
            nc.gpsimd.dma_start(c_dram[:], cexp[:])
            c_dram2 = dpool.tile([TILES, 1024], F32)

            # ---- bulk inputs, in tile-0 consumption order ----
            xs = cpool.tile([P, NCK, XW], BF16)
            wt = cpool.tile([P, NCK, K, COUT], BF16)
            XCUTS = [0, 130, 775, 1420, XW]  # first chunk small: tile 0 band
            xs_src = xs_d[:].rearrange("(ci p) t -> p ci t", p=P)
            wt_src = wt_d[:].rearrange("(ci p) k d -> p ci k d", p=P)
            nc.sync.dma_start(xs[:, :, 0:130], xs_src[:, :, 0:130])
            for k in range(K):
                nc.sync.dma_start(wt[:, :, k, :], wt_src[:, :, k, :])
            for lo, hi in zip(XCUTS[1:], XCUTS[2:]):
                nc.sync.dma_start(xs[:, :, lo:hi], xs_src[:, :, lo:hi])

            for t in range(TILES):
                o0 = t * OT

                # ---- Y[dp, k, d] for band p in [o0, o0+128) ----
                y_sb = ypool.tile([P, K, COUT], BF16, tag="y_sb")
                for k in range(K):
                    yp = psy.tile([P, COUT], F32, tag="yp")
                    for ci in range(NCK):
                        nc.tensor.matmul(
                            yp[:],
                            xs[:, ci, o0 : o0 + P],
                            wt[:, ci, k, :],
                            start=(ci == 0), stop=(ci == NCK - 1),
                        )
                    if k < 5:
                        nc.vector.tensor_copy(y_sb[:, k, :], yp[:])
                    else:
                        nc.scalar.copy(y_sb[:, k, :], yp[:])

                # ---- S_k[dp, o] = relu(1 - |(c_k - diag)|) ----
                # relayout this tile's c into one contiguous DRAM row, then a
                # 0-stride DMA expands it across all 128 partitions (4KB
                # contiguous per partition -> efficient descriptors). two
                # 484-wide regions of 4 k-blocks each (k3 in both).
                c_d3 = c_dram[:].rearrange("(b k) o -> b k o", b=1)
                c2_d3 = c_dram2[:].rearrange("(b t) f -> b t f", b=1)
                cb = cbpool.tile([P, 1024], F32, tag="cb")
                s_sb = stpool.tile([P, 1024], BF16, tag="s_sb")
                for roff, klo, nk in ((0, 0, 4), (512, 4, 3)):
                    nc.gpsimd.dma_start(
                        c2_d3[0:1, t, roff : roff + nk * OT].rearrange(
                            "b (k o) -> b k o", k=nk
                        ),
                        c_d3[0:1, klo : klo + nk, o0 : o0 + OT],
                    )
                nc.gpsimd.dma_start(
                    cb[:], c_dram2[t : t + 1, :].partition_broadcast(P)
                )
                reg = slice(0, 512 + 3 * OT)
                nc.vector.tensor_tensor(
                    cb[:, reg], cb[:, reg], diag7[:, reg],
                    mybir.AluOpType.subtract,
                )
                nc.scalar.activation(
                    cb[:, reg], cb[:, reg], mybir.ActivationFunctionType.Abs,
                )
                nc.scalar.activation(
                    s_sb[:, reg], cb[:, reg],
                    mybir.ActivationFunctionType.Relu,
                    bias=1.0, scale=-1.0,
                )

                # ---- out[o, d] = sum_k S_k^T Y_k + bias ----
                op = pso.tile([P, COUT], F32, tag="op")
                nc.tensor.matmul(
                    op[:OT], onesb[:, :OT], bias_sb[:],
                    start=True, stop=False,
                )
                for k in range(K):
                    koff = k * OT if k < 4 else 512 + (k - 4) * OT
                    nc.tensor.matmul(
                        op[:OT],
                        s_sb[:, koff : koff + OT],
                        y_sb[:, k, :],
                        start=False, stop=(k == K - 1),
                    )
                # evict [o, d] and store; host transposes to [d, o] on unshard
                o_sb = opool.tile([P, COUT], F32, tag="o_sb")
                if t % 2 == 0:
                    nc.scalar.copy(o_sb[:OT], op[:OT])
                else:
                    nc.vector.tensor_copy(o_sb[:OT], op[:OT])
                nc.sync.dma_start(out_d[o0 : o0 + OT, :], o_sb[:OT])

    nc.compile()
    return nc


def _install_axon_ntff_hook():
    """Provide antenv.axon_hooks (absent on this image) so that
    run_bass_kernel_spmd(trace=True) can capture NTFF profiles via the
    axon .so's C ABI.  Mirrors trn_agent_boot.trn_boot."""
    import contextlib
    import ctypes
    import types

    try:
        from antenv.axon_hooks import set_axon_ntff_profile_hook  # noqa: F401
        return
    except ImportError:
        pass

    so_path = "/opt/axon/libaxon_pjrt.so"
    if not os.path.exists(so_path):
        return
    lib = ctypes.CDLL(so_path)
    if not hasattr(lib, "axon_start_nrt_profile"):
        return
    lib.axon_start_nrt_profile.argtypes = [
        ctypes.POINTER(ctypes.c_int64), ctypes.c_size_t,
    ]
    lib.axon_start_nrt_profile.restype = ctypes.c_int64
    lib.axon_stop_nrt_profile.argtypes = [ctypes.c_char_p]
    lib.axon_stop_nrt_profile.restype = ctypes.c_int64

    @contextlib.contextmanager
    def _hook(output_dir, device_ids):
        import jax

        jax.devices()
        if device_ids:
            ids = (ctypes.c_int64 * len(device_ids))(*device_ids)
            rc = lib.axon_start_nrt_profile(ids, len(device_ids))
        else:
            rc = lib.axon_start_nrt_profile(None, 0)
        if rc != 0:
            raise RuntimeError(f"axon_start_nrt_profile rc={rc}")
        try:
            yield
        finally:
            n = lib.axon_stop_nrt_profile(str(output_dir).encode())
            print(f"ntff profile: {n} file(s) written to {output_dir}")

    box = {"h": _hook}
    mod = types.ModuleType("antenv.axon_hooks")
    mod.get_axon_ntff_profile_hook = lambda: box["h"]
    mod.set_axon_ntff_profile_hook = lambda h: box.__setitem__("h", h)
    import antenv

    sys.modules["antenv.axon_hooks"] = mod
    antenv.axon_hooks = mod

    # zero-egress env: skip the artifact upload in the trace path
    from concourse import bass_utils as _bu

    _bu.upload_artifacts = lambda d: f"local:{d}"


def _consts():
    # diag7[dp, koff(k)+j] = dp - j   (j = o - o0), bank-padded layout:
    # k<4 at k*OT, k>=4 at 512+(k-4)*OT
    dp = np.arange(P, dtype=np.float32).reshape(P, 1)
    j = np.arange(OT, dtype=np.float32).reshape(1, OT)
    blk = dp - j  # [P, OT]
    diag7 = np.zeros((P, 1024), dtype=np.float32)
    for i in range(4):
        diag7[:, i * OT : i * OT + OT] = blk
        diag7[:, 512 + i * OT : 512 + i * OT + OT] = blk
    kcol = np.arange(K, dtype=np.float32).reshape(K, 1).copy()
    ones = np.ones((1, P), dtype=np.float32)
    return diag7, kcol, ones


def kernel(x, offsets, weight, bias, _trace=False, _trace_kwargs=None):
    x = np.asarray(x, dtype=np.float32)
    offsets = np.asarray(offsets, dtype=np.float32)
    weight = np.asarray(weight, dtype=np.float32)
    bias = np.asarray(bias, dtype=np.float32)

    if "nc" not in _prog_cache:
        _prog_cache["nc"] = _build_program()
    nc = _prog_cache["nc"]

    w_t = np.ascontiguousarray(
        np.transpose(weight, (1, 2, 0)).astype(ml_dtypes.bfloat16)
    )  # [c, k, d]
    bias2 = np.ascontiguousarray(bias.reshape(1, COUT).astype(ml_dtypes.bfloat16))
    diag7, kcol, ones = _consts()

    in_maps = []
    for core in range(8):
        b, half = core // 2, core % 2
        o_off = half * HALF
        xs = np.zeros((CIN, XW), dtype=ml_dtypes.bfloat16)
        xw = min(L - o_off, XW)
        xs[:, :xw] = x[b][:, o_off : o_off + xw].astype(ml_dtypes.bfloat16)
        offsT = np.zeros((K, OPAD), dtype=np.float32)
        ow = min(OUT_LEN - o_off, OPAD)
        offsT[:, :ow] = offsets[b, 0, o_off : o_off + ow, :].T
        in_maps.append(
            {
                "xs": xs, "wt": w_t, "offsT": offsT, "bias2": bias2,
                "diag7": diag7, "kcol": kcol,
                "onesb": ones.astype(ml_dtypes.bfloat16),
            }
        )

    if _trace:
        _install_axon_ntff_hook()
    try:
        res = run_bass_kernel_spmd(
            nc, in_maps, core_ids=list(range(8)),
            trace=_trace, **(_trace_kwargs or {}),
        )
    except Exception:
        # transient runtime faults have been observed; one retry
        res = run_bass_kernel_spmd(
            nc, in_maps, core_ids=list(range(8)),
            trace=_trace, **(_trace_kwargs or {}),
        )

    out = np.empty((B, COUT, OUT_LEN), dtype=np.float32)
    for core in range(8):
        b, half = core // 2, core % 2
        o_off = half * HALF
        out[b, :, o_off : o_off + HALF] = res.results[core]["out"][:HALF, :].T
    if _trace:
        _prog_cache["last_exec_time_ns"] = res.exec_time_ns
    return out
